# revision 3
# baseline (speedup 1.0000x reference)
"""Trainium2 Bass kernel: GCN message passing (nn_DDI_gcn), 8 NeuronCores SPMD.

Math:
  agg[r] = sum_{e: row_idx[e]==r} vals[e] * mEmbed[col_idx[e] % 50000]
  out[i] = 2*(inter*relu(agg[i]) + (1-inter)*relu(agg[i+50000])),  i < 50000

Strategy (destination sharding; group-gather):
  * Core k owns output rows [6272k, 6272(k+1)) of each plane: 49 dest tiles
    of 128 rows. Host buckets edges per (core, dest tile).
  * The gather of embedding rows is descriptor-RATE-bound on HW (~10ns per
    descriptor regardless of 256B/512B/1KB payload), so each descriptor
    fetches a GROUP of G=4 adjacent table rows (1KB fp16) and edges are
    deduped at group granularity: distinct (tile, group) pairs are gathered
    once (~310K descriptors/core vs 400K edges/core).
  * Subrow j of a gathered group is addressed as a free-dim slice
    g_t[:, c, j*128:(j+1)*128] - no transposes. Each 128-slot chunk runs
    2G one-hot+matmul passes (G subrows x 2 planes): the one-hot selection
    matrix S[slot, d] = val*(iota==d) is built by one dual-op tensor_scalar
    from per-slot (d, val) scalars; TensorE accumulates S^T @ G into the
    plane's PSUM tile (the segment sum). Slots inactive in a pass carry V=0.
  * Group indices fit int16 (12500 < 32767): ONE gather call per tile, with
    trailing -1 padding skipped via a runtime count register.
  * Epilogue: a*relu(psA) + b*relu(psB) streamed out as f32.

Timing: per-call dispatch overhead in this environment is large (~60-90ms)
and drifts, so timed_run builds the body repeated 1x and kx inside one NEFF,
samples both interleaved, and reports the median of per-trial marginals.
"""

import numpy as np

import concourse.bass as bass
import concourse.bacc as bacc
import concourse.tile as tile
import concourse.mybir as mybir
from concourse.bass_utils import run_bass_kernel_spmd

MED = 50000
NCORES = 8
TILES = 49
RPC = TILES * 128
P = 128
F = 128
G = 4                       # table rows per gather descriptor

_NC_CACHE = {}


def build_nc(CGp, tiles=TILES, gbufs=3, repeat=1):
    NP = MED // G
    CAP = CGp * 128
    IC = CAP // 16
    GF = G * 128
    dt16 = mybir.dt.float16
    f32 = mybir.dt.float32

    nc = bacc.Bacc(None, target_bir_lowering=False)
    table = nc.dram_tensor("table", [NP, GF], dt16, kind="ExternalInput")
    idx_d = nc.dram_tensor("idx", [tiles, P, IC], mybir.dt.int16, kind="ExternalInput")
    dval_d = nc.dram_tensor("dval", [tiles, P, 4 * G * CGp], f32,
                            kind="ExternalInput")
    cnt_d = nc.dram_tensor("cnt", [1, tiles], mybir.dt.int32, kind="ExternalInput")
    ab_d = nc.dram_tensor("ab", [P, 2], f32, kind="ExternalInput")
    iota_d = nc.dram_tensor("iota", [P, P], dt16, kind="ExternalInput")
    out_d = nc.dram_tensor("out", [tiles, P, F], f32, kind="ExternalOutput")

    with tile.TileContext(nc) as tc:
        with (
            tc.tile_pool(name="const", bufs=1) as constp,
            tc.tile_pool(name="gbuf", bufs=gbufs) as gbufp,
            tc.tile_pool(name="meta", bufs=3) as metap,
            tc.tile_pool(name="sp", bufs=8) as sp,
            tc.tile_pool(name="ep", bufs=4) as ep,
            tc.tile_pool(name="psum", bufs=2, space=bass.MemorySpace.PSUM) as psp,
        ):
            iota_t = constp.tile([P, P], dt16, tag="iota")
            nc.sync.dma_start(iota_t[:], iota_d[:])
            ab_t = constp.tile([P, 2], f32, tag="ab")
            nc.sync.dma_start(ab_t[:], ab_d[:])
            cnt_t = constp.tile([1, tiles], mybir.dt.int32, tag="cnt")
            nc.sync.dma_start(cnt_t[:], cnt_d[:])
            r0 = nc.alloc_register(mybir.EngineType.Pool, "cnt0")

            for g in [g_ for _ in range(repeat) for g_ in range(tiles)]:
                idx_t = metap.tile([P, IC], mybir.dt.int16, tag="idx")
                nc.sync.dma_start(idx_t[:], idx_d[g])
                dv_t = metap.tile([P, 4 * G * CGp], f32, tag="dval")
                nc.sync.dma_start(dv_t[:], dval_d[g])

                g_t = gbufp.tile([P, CGp, GF], dt16, tag="g")
                nc.gpsimd.reg_load(r0, cnt_t[0:1, g : g + 1])
                nc.gpsimd.dma_gather(
                    g_t[:], table[:, :], idx_t[:], CAP, r0, GF,
                    single_packet=False,
                )

                psA = psp.tile([P, F], f32, tag="psA")
                psB = psp.tile([P, F], f32, tag="psB")
                for c in range(CGp):
                    for t in range(2 * G):
                        j, pl = t // 2, t % 2
                        s_t = sp.tile([P, P], dt16, tag="s")
                        dcol = (2 * t) * CGp + c
                        vcol = (2 * t + 1) * CGp + c
                        nc.vector.tensor_scalar(
                            s_t[:], iota_t[:],
                            dv_t[:, dcol : dcol + 1], dv_t[:, vcol : vcol + 1],
                            mybir.AluOpType.is_equal, mybir.AluOpType.mult,
                        )
                        ps = psA if pl == 0 else psB
                        nc.tensor.matmul(
                            ps[:], s_t[:], g_t[:, c, j * 128 : (j + 1) * 128],
                            start=(c == 0 and t == pl),
                            stop=(c == CGp - 1 and t == 2 * G - 2 + pl),
                        )

                t0 = ep.tile([P, F], f32, tag="t0")
                nc.vector.tensor_scalar(t0[:], psA[:], 0.0, ab_t[:, 0:1],
                                        mybir.AluOpType.max, mybir.AluOpType.mult)
                t1 = ep.tile([P, F], f32, tag="t1")
                nc.vector.tensor_scalar(t1[:], psB[:], 0.0, ab_t[:, 1:2],
                                        mybir.AluOpType.max, mybir.AluOpType.mult)
                o_t = ep.tile([P, F], f32, tag="o")
                nc.vector.tensor_tensor(o_t[:], t0[:], t1[:], mybir.AluOpType.add)
                nc.sync.dma_start(out_d[g], o_t[:])

    nc.compile()
    return nc


def preprocess(vals, mEmbed, inter, row_idx, col_idx, tiles=TILES, gbufs=3):
    E = row_idx.shape[0]
    col = col_idx.astype(np.int64) % MED
    rowl = row_idx.astype(np.int64)
    plane = rowl // MED
    prow = rowl % MED
    core = np.minimum(prow // RPC, NCORES - 1)
    lt = (prow - core * RPC) >> 7
    d = (prow & 127).astype(np.float32)
    q = col // G                                   # group index
    j = col % G                                    # subrow within group
    NP = MED // G

    call = core * tiles + lt                       # one gather call per (core,tile)
    ncalls = NCORES * tiles

    # slot key = (call, q, occ): occ = rank within (call, q, j, plane) so one
    # slot carries at most one edge per (subrow, plane) pass.
    okey = ((call * NP + q) * G + j) * 2 + plane
    oorder = np.argsort(okey, kind="stable")
    osort = okey[oorder]
    grp_start = np.concatenate([[True], osort[1:] != osort[:-1]])
    gid = np.cumsum(grp_start) - 1
    gstarts = np.flatnonzero(grp_start)
    occ = np.arange(E) - gstarts[gid]
    assert occ.max() < 64
    occ_u = np.empty(E, np.int64)
    occ_u[oorder] = occ
    skey = (call * NP + q) * 64 + occ_u
    uniq, slot_of_edge = np.unique(skey, return_inverse=True)
    call_of_slot = (uniq // 64) // NP
    cnt = np.bincount(call_of_slot, minlength=ncalls)

    CGp = max(1, int(np.ceil(cnt.max() / 128)))
    CAP = CGp * 128
    TOT = ncalls * CAP
    starts = np.concatenate([[0], np.cumsum(cnt)[:-1]])
    nslots = len(uniq)
    srank = np.arange(nslots, dtype=np.int64) - starts[call_of_slot]
    slot_pos = call_of_slot * CAP + srank
    slot = slot_pos[slot_of_edge]

    IDX = np.full(TOT, -1, np.int16)
    IDX[slot_pos] = ((uniq // 64) % NP).astype(np.int16)

    counts = cnt.reshape(NCORES, tiles).astype(np.int32)
    empty = counts == 0
    if empty.any():
        base = np.arange(ncalls).reshape(NCORES, tiles) * CAP
        IDX[base[empty]] = 0
        counts = np.maximum(counts, 1)

    # First `gbufs` tiles gather full capacity so rotating pool buffers never
    # expose uninitialized SBUF (NaN x 0 would poison psum).
    IDXv = IDX.reshape(NCORES, tiles, CAP)
    IDXv[:, :gbufs, :][IDXv[:, :gbufs, :] < 0] = 0
    counts[:, :gbufs] = CAP

    DT = np.zeros((2 * G, TOT), np.float32)
    VT = np.zeros((2 * G, TOT), np.float32)
    tix = (j * 2 + plane).astype(np.int64)
    DT[tix, slot] = d
    VT[tix, slot] = np.asarray(vals, np.float32)

    # gather idx stream order: slot i -> partition i%16, word i//16
    idx16 = IDX.reshape(NCORES, tiles, CAP // 16, 16).transpose(0, 1, 3, 2)
    idx128 = np.ascontiguousarray(np.tile(idx16, (1, 1, 8, 1)))

    def chunked(X):
        return X.reshape(NCORES, tiles, CGp, 128).transpose(0, 1, 3, 2)

    sections = []
    for t in range(2 * G):
        sections.append(chunked(DT[t]))
        sections.append(chunked(VT[t]))
    dval = np.ascontiguousarray(np.concatenate(sections, axis=3),
                                dtype=np.float32)

    table16 = np.asarray(mEmbed, np.float32).astype(np.float16)
    tableG = np.ascontiguousarray(table16.reshape(NP, G * F))
    iota = np.ascontiguousarray(
        np.broadcast_to(np.arange(128, dtype=np.float16), (128, 128)))
    a = 2.0 * np.float32(np.asarray(inter).reshape(-1)[0])
    b = np.float32(2.0) - a
    ab = np.ascontiguousarray(
        np.stack([np.full(128, a, np.float32), np.full(128, b, np.float32)],
                 axis=1))
    cnts = np.ascontiguousarray(counts.reshape(NCORES, 1, tiles))
    return CGp, tableG, iota, ab, idx128, dval, cnts


def _in_maps(per):
    CGp, tableG, iota, ab, idx128, dval, cnts = per
    return [
        {"table": tableG, "iota": iota, "ab": ab,
         "idx": idx128[k], "dval": dval[k], "cnt": cnts[k]}
        for k in range(NCORES)
    ]


def _run(vals, mEmbed, inter, row_idx, col_idx, trace=False):
    per = preprocess(vals, mEmbed, inter, row_idx, col_idx)
    CGp = per[0]
    key = (G, CGp, 1)
    if key not in _NC_CACHE:
        _NC_CACHE[key] = build_nc(CGp)
    nc = _NC_CACHE[key]
    res = run_bass_kernel_spmd(nc, _in_maps(per), core_ids=list(range(NCORES)),
                               trace=trace)
    full = np.concatenate(
        [res.results[k]["out"].reshape(RPC, F) for k in range(NCORES)], axis=0)
    return np.ascontiguousarray(full[:MED]), res


def kernel(vals, mEmbed, inter, row_idx, col_idx):
    out, _ = _run(vals, mEmbed, inter, row_idx, col_idx, trace=False)
    return out


def _make_sharded(nc, donate=False):
    """Replicate bass2jax.run_bass_via_pjrt's executable construction so we
    can reuse it for repeated timed executions."""
    import jax
    from jax.sharding import Mesh, PartitionSpec
    from jax.experimental.shard_map import shard_map
    from concourse import bass2jax as b2j

    b2j.install_neuronx_cc_hook()
    partition_name = nc.partition_id_tensor.name if nc.partition_id_tensor else None
    in_names, out_names, out_avals, zero_outs = [], [], [], []
    for alloc in nc.m.functions[0].allocations:
        if not isinstance(alloc, mybir.MemoryLocationSet):
            continue
        name = alloc.memorylocations[0].name
        if alloc.kind == "ExternalInput":
            if name != partition_name:
                in_names.append(name)
        elif alloc.kind == "ExternalOutput":
            out_names.append(name)
            shape = tuple(alloc.tensor_shape)
            dtype = mybir.dt.np(alloc.dtype)
            out_avals.append(jax.core.ShapedArray(shape, dtype))
            zero_outs.append(np.zeros(shape, dtype))
    n_params = len(in_names)
    in_names = in_names + out_names
    if partition_name is not None:
        in_names = in_names + [partition_name]

    def _body(*args):
        operands = list(args)
        if partition_name is not None:
            operands.append(b2j.partition_id_tensor())
        outs = b2j._bass_exec_p.bind(
            *operands,
            out_avals=tuple(out_avals),
            in_names=tuple(in_names),
            out_names=tuple(out_names),
            lowering_input_output_aliases=(),
            sim_require_finite=True,
            sim_require_nnan=True,
            nc=nc,
        )
        return tuple(outs)

    devices = jax.devices()[:NCORES]
    mesh = Mesh(np.asarray(devices), ("core",))
    in_specs = (PartitionSpec("core"),) * (n_params + len(out_names))
    out_specs = (PartitionSpec("core"),) * len(out_names)
    kw = dict(donate_argnums=tuple(range(n_params, n_params + len(out_names)))) if donate else {}

    sharded = jax.jit(
        shard_map(_body, mesh=mesh, in_specs=in_specs,
                  out_specs=out_specs, check_rep=False),
        keep_unused=True, **kw)
    return sharded, mesh, in_names[:n_params], out_names, zero_outs


def timed_run(vals, mEmbed, inter, row_idx, col_idx, k=4, samples=12,
              build_kwargs=None):
    """Marginal HW time: body repeated 1x and kx inside one NEFF;
    (T(k)-T(1))/(k-1), median over interleaved trials (dispatch overhead
    here is ~60-90ms and drifts, so single-shot timing is meaningless)."""
    import time
    import jax
    from jax.sharding import NamedSharding, PartitionSpec

    per = preprocess(vals, mEmbed, inter, row_idx, col_idx)
    CGp = per[0]
    bk = dict(build_kwargs or {})
    per_core = _in_maps(per)

    def build(repeat):
        ck = (G, CGp, repeat, tuple(sorted(bk.items())))
        if ck not in _NC_CACHE:
            _NC_CACHE[ck] = build_nc(CGp, repeat=repeat, **bk)
        nc = _NC_CACHE[ck]
        sharded, mesh, in_names, out_names, zero_outs = _make_sharded(nc)
        sh = NamedSharding(mesh, PartitionSpec("core"))
        concat_in = [
            jax.device_put(
                np.concatenate([np.asarray(per_core[c][n]) for c in range(NCORES)],
                               axis=0), sh)
            for n in in_names
        ]
        concat_zero = [
            jax.device_put(np.zeros((NCORES * z.shape[0], *z.shape[1:]), z.dtype), sh)
            for z in zero_outs
        ]

        def run():
            out = sharded(*concat_in, *concat_zero)
            jax.block_until_ready(out)

        run()  # warm up executable + buffers
        return run

    run1 = build(1)
    runk = build(k)
    diffs, t1s, tks = [], [], []
    for _ in range(samples):
        t0 = time.perf_counter()
        run1()
        t1 = time.perf_counter()
        runk()
        t2 = time.perf_counter()
        t1s.append(t1 - t0)
        tks.append(t2 - t1)
        diffs.append(((t2 - t1) - (t1 - t0)) / (k - 1))
    diffs.sort()
    n = len(diffs)
    med = (diffs[(n - 1) // 2] + diffs[n // 2]) / 2
    return int(med * 1e9), int(min(t1s) * 1e9), int(min(tks) * 1e9)


# revision 5
# speedup vs baseline: 1.1987x; 1.1987x over previous
"""Trainium2 Bass kernel: GCN message passing (nn_DDI_gcn), 8 NeuronCores SPMD.

G=4 group-gather (1KB descriptors, group-dedupe, runtime-count pad skip) with
plane-merged 256-wide one-hot passes and an fp32 iota input (16-bit DVE ops
engage 2-port perf mode, locking GpSimd/SWDGE out of its shared SBUF port and
serializing the gather against compute). See git-less work/ history: this is
v7 at G=4. timed_run reports the median of interleaved repeat-marginals.
"""
import numpy as np

import concourse.bass as bass
import concourse.bacc as bacc
import concourse.tile as tile
import concourse.mybir as mybir
from concourse.bass_utils import run_bass_kernel_spmd

MED = 50000
NCORES = 8
TILES = 49
RPC = TILES * 128
P = 128
F = 128

GROUP = 4
_NC_CACHE = {}


def build_nc(CGp, G=GROUP, tiles=TILES, gbufs=2, repeat=1):
    NP = MED // G
    CAP = CGp * 128
    IC = CAP // 16
    GF = G * 128
    dt16 = mybir.dt.float16
    f32 = mybir.dt.float32

    nc = bacc.Bacc(None, target_bir_lowering=False)
    table = nc.dram_tensor("table", [NP, GF], dt16, kind="ExternalInput")
    idx_d = nc.dram_tensor("idx", [tiles, P, IC], mybir.dt.int16, kind="ExternalInput")
    dval_d = nc.dram_tensor("dval", [tiles, P, 2 * G * CGp], f32,
                            kind="ExternalInput")
    cnt_d = nc.dram_tensor("cnt", [1, tiles], mybir.dt.int32, kind="ExternalInput")
    ab_d = nc.dram_tensor("ab", [P, 2], f32, kind="ExternalInput")
    iota_d = nc.dram_tensor("iota", [P, 2 * P], f32, kind="ExternalInput")
    out_d = nc.dram_tensor("out", [tiles, P, F], f32, kind="ExternalOutput")

    with tile.TileContext(nc) as tc:
        with (
            tc.tile_pool(name="const", bufs=1) as constp,
            tc.tile_pool(name="gbuf", bufs=gbufs) as gbufp,
            tc.tile_pool(name="meta", bufs=3) as metap,
            tc.tile_pool(name="sp", bufs=8) as sp,
            tc.tile_pool(name="ep", bufs=4) as ep,
            tc.tile_pool(name="psum", bufs=2, space=bass.MemorySpace.PSUM) as psp,
        ):
            iota_t = constp.tile([P, 2 * P], f32, tag="iota")
            nc.sync.dma_start(iota_t[:], iota_d[:])
            ab_t = constp.tile([P, 2], f32, tag="ab")
            nc.sync.dma_start(ab_t[:], ab_d[:])
            cnt_t = constp.tile([1, tiles], mybir.dt.int32, tag="cnt")
            nc.sync.dma_start(cnt_t[:], cnt_d[:])
            r0 = nc.alloc_register(mybir.EngineType.Pool, "cnt0")

            for g in [g_ for _ in range(repeat) for g_ in range(tiles)]:
                idx_t = metap.tile([P, IC], mybir.dt.int16, tag="idx")
                nc.sync.dma_start(idx_t[:], idx_d[g])
                dv_t = metap.tile([P, 2 * G * CGp], f32, tag="dval")
                nc.sync.dma_start(dv_t[:], dval_d[g])

                g_t = gbufp.tile([P, CGp, GF], dt16, tag="g")
                nc.gpsimd.reg_load(r0, cnt_t[0:1, g : g + 1])
                nc.gpsimd.dma_gather(
                    g_t[:], table[:, :], idx_t[:], CAP, r0, GF,
                    single_packet=False,
                )

                psA = psp.tile([P, F], f32, tag="psA")
                psB = psp.tile([P, F], f32, tag="psB")
                for c in range(CGp):
                    for t in range(G):
                        s_t = sp.tile([P, 2 * P], dt16, tag="s")
                        dcol = (2 * t) * CGp + c
                        vcol = (2 * t + 1) * CGp + c
                        nc.vector.tensor_scalar(
                            s_t[:], iota_t[:],
                            dv_t[:, dcol : dcol + 1], dv_t[:, vcol : vcol + 1],
                            mybir.AluOpType.is_equal, mybir.AluOpType.mult,
                        )
                        mov = g_t[:, c, t * 128 : (t + 1) * 128]
                        first = c == 0 and t == 0
                        last = c == CGp - 1 and t == G - 1
                        nc.tensor.matmul(psA[:], s_t[:, 0:P], mov,
                                         start=first, stop=last)
                        nc.tensor.matmul(psB[:], s_t[:, P : 2 * P], mov,
                                         start=first, stop=last)

                t0 = ep.tile([P, F], f32, tag="t0")
                nc.vector.tensor_scalar(t0[:], psA[:], 0.0, ab_t[:, 0:1],
                                        mybir.AluOpType.max, mybir.AluOpType.mult)
                t1 = ep.tile([P, F], f32, tag="t1")
                nc.vector.tensor_scalar(t1[:], psB[:], 0.0, ab_t[:, 1:2],
                                        mybir.AluOpType.max, mybir.AluOpType.mult)
                o_t = ep.tile([P, F], f32, tag="o")
                nc.vector.tensor_tensor(o_t[:], t0[:], t1[:], mybir.AluOpType.add)
                nc.sync.dma_start(out_d[g], o_t[:])

    nc.compile()
    return nc


def preprocess(vals, mEmbed, inter, row_idx, col_idx, G=GROUP, tiles=TILES,
                gbufs=2):
    E = row_idx.shape[0]
    col = col_idx.astype(np.int64) % MED
    rowl = row_idx.astype(np.int64)
    plane = rowl // MED
    prow = rowl % MED
    core = np.minimum(prow // RPC, NCORES - 1)
    lt = (prow - core * RPC) >> 7
    d = (prow & 127).astype(np.float32)
    dcode = d + 128.0 * plane.astype(np.float32)   # one-hot over [0, 256)
    q = col // G
    j = col % G
    NP = MED // G

    call = core * tiles + lt
    ncalls = NCORES * tiles

    # slot key = (call, q, occ): occ = rank within (call, q, j) -- one edge
    # per (slot, subrow) since each subrow has a single merged pass.
    okey = (call * NP + q) * G + j
    oorder = np.argsort(okey, kind="stable")
    osort = okey[oorder]
    grp_start = np.concatenate([[True], osort[1:] != osort[:-1]])
    gid = np.cumsum(grp_start) - 1
    gstarts = np.flatnonzero(grp_start)
    occ = np.arange(E) - gstarts[gid]
    assert occ.max() < 64
    occ_u = np.empty(E, np.int64)
    occ_u[oorder] = occ
    skey = (call * NP + q) * 64 + occ_u
    uniq, slot_of_edge = np.unique(skey, return_inverse=True)
    call_of_slot = (uniq // 64) // NP
    cnt = np.bincount(call_of_slot, minlength=ncalls)

    CGp = max(1, int(np.ceil(cnt.max() / 128)))
    CAP = CGp * 128
    TOT = ncalls * CAP
    starts = np.concatenate([[0], np.cumsum(cnt)[:-1]])
    nslots = len(uniq)
    srank = np.arange(nslots, dtype=np.int64) - starts[call_of_slot]
    slot_pos = call_of_slot * CAP + srank
    slot = slot_pos[slot_of_edge]

    IDX = np.full(TOT, -1, np.int16)
    IDX[slot_pos] = ((uniq // 64) % NP).astype(np.int16)

    counts = cnt.reshape(NCORES, tiles).astype(np.int32)
    empty = counts == 0
    if empty.any():
        base = np.arange(ncalls).reshape(NCORES, tiles) * CAP
        IDX[base[empty]] = 0
        counts = np.maximum(counts, 1)

    IDXv = IDX.reshape(NCORES, tiles, CAP)
    IDXv[:, :gbufs, :][IDXv[:, :gbufs, :] < 0] = 0
    counts[:, :gbufs] = CAP

    DT = np.zeros((G, TOT), np.float32)
    VT = np.zeros((G, TOT), np.float32)
    DT[:] = -1.0                                   # no iota value matches -1
    DT[j, slot] = dcode
    VT[j, slot] = np.asarray(vals, np.float32)

    idx16 = IDX.reshape(NCORES, tiles, CAP // 16, 16).transpose(0, 1, 3, 2)
    idx128 = np.ascontiguousarray(np.tile(idx16, (1, 1, 8, 1)))

    def chunked(X):
        return X.reshape(NCORES, tiles, CGp, 128).transpose(0, 1, 3, 2)

    sections = []
    for t in range(G):
        sections.append(chunked(DT[t]))
        sections.append(chunked(VT[t]))
    dval = np.ascontiguousarray(np.concatenate(sections, axis=3),
                                dtype=np.float32)

    table16 = np.asarray(mEmbed, np.float32).astype(np.float16)
    tableG = np.ascontiguousarray(table16.reshape(NP, G * F))
    iota = np.ascontiguousarray(
        np.broadcast_to(np.arange(256, dtype=np.float32), (128, 256)))
    a = 2.0 * np.float32(np.asarray(inter).reshape(-1)[0])
    b = np.float32(2.0) - a
    ab = np.ascontiguousarray(
        np.stack([np.full(128, a, np.float32), np.full(128, b, np.float32)],
                 axis=1))
    cnts = np.ascontiguousarray(counts.reshape(NCORES, 1, tiles))
    return CGp, tableG, iota, ab, idx128, dval, cnts


def _run(vals, mEmbed, inter, row_idx, col_idx, G=GROUP, trace=False):
    CGp, tableG, iota, ab, idx128, dval, cnts = preprocess(
        vals, mEmbed, inter, row_idx, col_idx, G=G)
    key = ("v7", G, CGp, 1)
    if key not in _NC_CACHE:
        _NC_CACHE[key] = build_nc(CGp, G=G)
    nc = _NC_CACHE[key]
    in_maps = [
        {"table": tableG, "iota": iota, "ab": ab,
         "idx": idx128[k], "dval": dval[k], "cnt": cnts[k]}
        for k in range(NCORES)
    ]
    res = run_bass_kernel_spmd(nc, in_maps, core_ids=list(range(NCORES)),
                               trace=trace)
    full = np.concatenate(
        [res.results[k]["out"].reshape(RPC, F) for k in range(NCORES)], axis=0)
    return np.ascontiguousarray(full[:MED]), res


def kernel(vals, mEmbed, inter, row_idx, col_idx, G=GROUP):
    out, _ = _run(vals, mEmbed, inter, row_idx, col_idx, G=G)
    return out


def _in_maps(per):
    CGp, tableG, iota, ab, idx128, dval, cnts = per
    return [
        {"table": tableG, "iota": iota, "ab": ab,
         "idx": idx128[k], "dval": dval[k], "cnt": cnts[k]}
        for k in range(NCORES)
    ]

def _make_sharded(nc, donate=False):
    """Replicate bass2jax.run_bass_via_pjrt's executable construction so we
    can reuse it for repeated timed executions."""
    import jax
    from jax.sharding import Mesh, PartitionSpec
    from jax.experimental.shard_map import shard_map
    from concourse import bass2jax as b2j

    b2j.install_neuronx_cc_hook()
    partition_name = nc.partition_id_tensor.name if nc.partition_id_tensor else None
    in_names, out_names, out_avals, zero_outs = [], [], [], []
    for alloc in nc.m.functions[0].allocations:
        if not isinstance(alloc, mybir.MemoryLocationSet):
            continue
        name = alloc.memorylocations[0].name
        if alloc.kind == "ExternalInput":
            if name != partition_name:
                in_names.append(name)
        elif alloc.kind == "ExternalOutput":
            out_names.append(name)
            shape = tuple(alloc.tensor_shape)
            dtype = mybir.dt.np(alloc.dtype)
            out_avals.append(jax.core.ShapedArray(shape, dtype))
            zero_outs.append(np.zeros(shape, dtype))
    n_params = len(in_names)
    in_names = in_names + out_names
    if partition_name is not None:
        in_names = in_names + [partition_name]

    def _body(*args):
        operands = list(args)
        if partition_name is not None:
            operands.append(b2j.partition_id_tensor())
        outs = b2j._bass_exec_p.bind(
            *operands,
            out_avals=tuple(out_avals),
            in_names=tuple(in_names),
            out_names=tuple(out_names),
            lowering_input_output_aliases=(),
            sim_require_finite=True,
            sim_require_nnan=True,
            nc=nc,
        )
        return tuple(outs)

    devices = jax.devices()[:NCORES]
    mesh = Mesh(np.asarray(devices), ("core",))
    in_specs = (PartitionSpec("core"),) * (n_params + len(out_names))
    out_specs = (PartitionSpec("core"),) * len(out_names)
    kw = dict(donate_argnums=tuple(range(n_params, n_params + len(out_names)))) if donate else {}

    sharded = jax.jit(
        shard_map(_body, mesh=mesh, in_specs=in_specs,
                  out_specs=out_specs, check_rep=False),
        keep_unused=True, **kw)
    return sharded, mesh, in_names[:n_params], out_names, zero_outs


def timed_run(vals, mEmbed, inter, row_idx, col_idx, k=4, samples=12,
              build_kwargs=None):
    """Marginal HW time: body repeated 1x and kx inside one NEFF;
    (T(k)-T(1))/(k-1), median over interleaved trials (dispatch overhead
    here is ~60-90ms and drifts, so single-shot timing is meaningless)."""
    import time
    import jax
    from jax.sharding import NamedSharding, PartitionSpec

    per = preprocess(vals, mEmbed, inter, row_idx, col_idx)
    CGp = per[0]
    bk = dict(build_kwargs or {})
    per_core = _in_maps(per)

    def build(repeat):
        ck = ("m", GROUP, CGp, repeat, tuple(sorted(bk.items())))
        if ck not in _NC_CACHE:
            _NC_CACHE[ck] = build_nc(CGp, repeat=repeat, **bk)
        nc = _NC_CACHE[ck]
        sharded, mesh, in_names, out_names, zero_outs = _make_sharded(nc)
        sh = NamedSharding(mesh, PartitionSpec("core"))
        concat_in = [
            jax.device_put(
                np.concatenate([np.asarray(per_core[c][n]) for c in range(NCORES)],
                               axis=0), sh)
            for n in in_names
        ]
        concat_zero = [
            jax.device_put(np.zeros((NCORES * z.shape[0], *z.shape[1:]), z.dtype), sh)
            for z in zero_outs
        ]

        def run():
            out = sharded(*concat_in, *concat_zero)
            jax.block_until_ready(out)

        run()  # warm up executable + buffers
        return run

    run1 = build(1)
    runk = build(k)
    diffs, t1s, tks = [], [], []
    for _ in range(samples):
        t0 = time.perf_counter()
        run1()
        t1 = time.perf_counter()
        runk()
        t2 = time.perf_counter()
        t1s.append(t1 - t0)
        tks.append(t2 - t1)
        diffs.append(((t2 - t1) - (t1 - t0)) / (k - 1))
    diffs.sort()
    n = len(diffs)
    med = (diffs[(n - 1) // 2] + diffs[n // 2]) / 2
    return int(med * 1e9), int(min(t1s) * 1e9), int(min(tks) * 1e9)
